# revision 1
# baseline (speedup 1.0000x reference)
"""Trainium2 Bass kernel for causal self-attention with GQA + RoPE.

Model: B=2, T=2048, C=2048, H=16 query heads, H_KV=4 kv heads, D=128.

Sharding (8 NeuronCores, pure SPMD, no collectives):
  core i -> batch b = i // 4, kv-group g = i % 4
            (query heads 4g..4g+3, kv head g, all T positions of batch b).
  Every core runs an identical program; only input data differs.
  o_proj is computed against the row-slice wo[512g:512(g+1), :], giving a
  partial [T, C] output per core; the sum over the 4 cores of each batch
  (the tensor-parallel all-reduce) is done on the host in numpy.

v2 (bf16): all matmul operands are bf16 (PSUM accumulation stays fp32).
  - bf16 weights get fast-weight-load (4x) so LDWEIGHTS fully hides under
    the matmul stream; fp32r LDW (191ns) was partially exposed (263ns/MM
    measured vs 216ns ideal).
  - halves DMA traffic (x, weights in bf16; out stored bf16, host casts).
  - causal diagonal narrowing: for the 4 diagonal 128-wide k-subtiles of
    each 512-wide q-chunk, S/PV/rowsum matmuls only cover q >= 128m
    (saves ~15us of PE time); a single upper-triangular [128,128] mask
    handles the remaining in-tile triangle.
  - o_proj matmuls of q-chunk aq-1 are interleaved as filler between
    attention groups of chunk aq so the PE never waits on the exp->PV
    dependency; S matmuls also run 2 groups ahead.
  - wo is preloaded to SBUF once (16KB/partition) instead of streamed 4x.
"""

import math
import os

import numpy as np

os.environ.setdefault("MYCRO_LOCAL_CACHE", "1")

P = 128
D = 128
H = 16
H_KV = 4
GQ = H // H_KV  # 4 query heads per kv head (= per core)
B = 2
T_FULL = 2048
C_DIM = 2048
NCORES = 8
ROPE_BASE = 10000.0


def _rope_tables(T):
    inv_freq = 1.0 / (ROPE_BASE ** (np.arange(0, D, 2, dtype=np.float32) / D))
    t = np.arange(T, dtype=np.float32)
    freqs = np.outer(t, inv_freq)  # [T, D/2]
    emb = np.concatenate((freqs, freqs), axis=-1)  # [T, D]
    return (
        np.ascontiguousarray(np.cos(emb).T.astype(np.float32)),  # [D, T]
        np.ascontiguousarray(np.sin(emb).T.astype(np.float32)),
    )


def _rot_lhsT():
    # rotate_half(q) = R @ q with R[d, d+64] = -1 (d < 64), R[d, d-64] = +1.
    # matmul computes lhsT.T @ rhs, so pass lhsT = R^T.
    R = np.zeros((D, D), dtype=np.float32)
    half = D // 2
    R[np.arange(half), np.arange(half) + half] = -1.0
    R[np.arange(half) + half, np.arange(half)] = 1.0
    return np.ascontiguousarray(R.T)


def _tri128():
    # tri[k, j] = 1 if j >= k else 0: the in-subtile causal triangle after
    # diagonal narrowing (column j of a narrowed diag slice is q = 128m + j,
    # row k is k_local; valid iff j >= k).
    k = np.arange(P)
    return (k[None, :] >= k[:, None]).astype(np.float32)


def build_nc(T=T_FULL):
    """Build the per-core Bass/Tile program (identical across cores)."""
    from contextlib import ExitStack

    import concourse.mybir as mybir
    import concourse.tile as tile
    from concourse import bacc
    from concourse.masks import make_identity

    f32 = mybir.dt.float32
    bf16 = mybir.dt.bfloat16
    Exp = mybir.ActivationFunctionType.Exp
    MULT = mybir.AluOpType.mult
    ADD = mybir.AluOpType.add
    SCALE = 1.0 / math.sqrt(D)

    NCC = C_DIM // P  # 16 contraction chunks
    NQC = T // 512  # projection / attention q-chunks (512-wide)
    NCT = C_DIM // 512  # o_proj column tiles
    NKB = T // P  # 128-wide k subtiles

    nc = bacc.Bacc(
        "TRN2",
        target_bir_lowering=False,
        debug=False,
        num_devices=NCORES,
    )

    xt = nc.dram_tensor("xt", [C_DIM, T], bf16, kind="ExternalInput").ap()
    wq = nc.dram_tensor("wq", [C_DIM, GQ * D], bf16, kind="ExternalInput").ap()
    wk = nc.dram_tensor("wk", [C_DIM, D], bf16, kind="ExternalInput").ap()
    wv = nc.dram_tensor("wv", [C_DIM, D], bf16, kind="ExternalInput").ap()
    wo = nc.dram_tensor("wo", [GQ * D, C_DIM], bf16, kind="ExternalInput").ap()
    cosT = nc.dram_tensor("cosT", [D, T], bf16, kind="ExternalInput").ap()
    sinT = nc.dram_tensor("sinT", [D, T], bf16, kind="ExternalInput").ap()
    trim = nc.dram_tensor("trim", [P, P], bf16, kind="ExternalInput").ap()
    onesm = nc.dram_tensor("onesm", [P, P], bf16, kind="ExternalInput").ap()
    rotm = nc.dram_tensor("rotm", [P, P], bf16, kind="ExternalInput").ap()
    out = nc.dram_tensor("out", [T, C_DIM], bf16, kind="ExternalOutput").ap()

    with tile.TileContext(nc) as tc, ExitStack() as ctx:
        const = ctx.enter_context(tc.tile_pool(name="const", bufs=1))
        acts = ctx.enter_context(tc.tile_pool(name="acts", bufs=1))

        wq_r = wq.rearrange("(cc p) n -> p cc n", p=P)
        wk_r = wk.rearrange("(cc p) n -> p cc n", p=P)
        wv_r = wv.rearrange("(cc p) n -> p cc n", p=P)
        xt_r = xt.rearrange("(cc p) t -> p cc t", p=P)
        wo_r = wo.rearrange("(h p) (ct n) -> p h ct n", p=P, n=512)

        ones_sb = const.tile([P, P], bf16)
        rot_sb = const.tile([P, P], bf16)
        ident = const.tile([P, P], bf16)
        tri_sb = const.tile([P, P], bf16)

        # long-lived activations (all bf16: 44KB/partition total)
        qt_sb = [acts.tile([P, T], bf16, name=f"qt{h}") for h in range(GQ)]
        kt_sb = acts.tile([P, T], bf16, name="kt")
        v_sb = acts.tile([P, NKB, D], bf16, name="vnat")
        y_sb = [acts.tile([P, T], bf16, name=f"yt{h}") for h in range(GQ)]
        wo_sb = acts.tile([P, GQ, NCT, 512], bf16, name="wo_sb")

        # ---------------- phase 1: projections + rope ----------------
        with (
            tc.tile_pool(name="pwts", bufs=1) as wpool,
            tc.tile_pool(name="xts", bufs=4) as xt_pool,
            tc.tile_pool(name="rope_t", bufs=1) as rope_pool,
            tc.tile_pool(name="proj_ps", bufs=1, space="PSUM") as proj_ps,
            tc.tile_pool(name="aux_ps", bufs=1, space="PSUM") as aux_ps,
            tc.tile_pool(name="ptmp", bufs=2) as ptmp,
        ):
            # weight tiles: per-cc DMAs so the first projection matmul can
            # start as soon as chunk 0 lands. Queue plan (per-queue ~90GB/s):
            #   sync:   x chunk0 cc0-11, then x chunks 1-3 (xg 0,2)
            #   scalar: wq (all 16 cc), then x chunks 1-3 (xg 1,3)
            #   gpsimd: wk/wv, x chunk0 cc12-15, cos/sin, consts, wo
            wq_sb = wpool.tile([P, NCC, GQ * D], bf16)
            wk_sb = wpool.tile([P, NCC, D], bf16)
            wv_sb = wpool.tile([P, NCC, D], bf16)
            lead_xs = xt_pool.tile([P, NCC, 512], bf16, tag="xlead", name="lead_xs")
            cos_sb = rope_pool.tile([P, T], bf16)
            sin_sb = rope_pool.tile([P, T], bf16)
            for cc in range(NCC):
                nc.scalar.dma_start(wq_sb[:, cc, :], wq_r[:, cc, :])
            # gpsimd's software DGE issues ~1 dma_start / 630ns, so order by
            # deadline: wk0/wv0 (first matmul), x cc12-15 (consumed ~t+18us),
            # then the rest of wk/wv, consts, and the wo preload last.
            nc.gpsimd.dma_start(wk_sb[:, 0, :], wk_r[:, 0, :])
            nc.gpsimd.dma_start(wv_sb[:, 0, :], wv_r[:, 0, :])
            for cc in range(10):
                nc.sync.dma_start(lead_xs[:, cc, :], xt_r[:, cc, 0:512])
            for cc in range(10, NCC):
                nc.gpsimd.dma_start(lead_xs[:, cc, :], xt_r[:, cc, 0:512])
            for cc in range(1, NCC):
                nc.gpsimd.dma_start(wk_sb[:, cc, :], wk_r[:, cc, :])
                nc.gpsimd.dma_start(wv_sb[:, cc, :], wv_r[:, cc, :])
            nc.gpsimd.dma_start(ones_sb[:], onesm)
            nc.gpsimd.dma_start(rot_sb[:], rotm)
            nc.gpsimd.dma_start(tri_sb[:], trim)
            make_identity(nc, ident)
            # wo preload (needed only once attention chunk 0 finishes)
            for h in range(GQ):
                for ct in range(NCT):
                    nc.gpsimd.dma_start(wo_sb[:, h, ct, :], wo_r[:, h, ct, :])
            # cos/sin per-chunk slices on sync, right behind chunk-0 x, so
            # chunk 0's rope tables land before its projections finish.
            for c in range(NQC):
                nc.sync.dma_start(cos_sb[:, 512 * c : 512 * (c + 1)],
                                  cosT[:, 512 * c : 512 * (c + 1)])
                nc.sync.dma_start(sin_sb[:, 512 * c : 512 * (c + 1)],
                                  sinT[:, 512 * c : 512 * (c + 1)])
            # warm the ACT exp table set during the initial DMA wait
            warm = ptmp.tile([P, 1], f32, name="warm", tag="warm")
            nc.scalar.activation(warm[:], warm[:], Exp)

            XG = 2  # xt c-chunks per streamed tile (chunks 1..3)
            for qc in range(NQC):
                q0 = qc * 512
                if qc == 0:
                    xt_tiles = [lead_xs[:, xg * XG : (xg + 1) * XG, :]
                                for xg in range(NCC // XG)]
                else:
                    xt_tiles = []
                    for xg in range(NCC // XG):
                        xs = xt_pool.tile([P, XG, 512], bf16, tag="xt", name="xs")
                        q_ = nc.sync if xg % 2 == 0 else nc.scalar
                        q_.dma_start(
                            xs[:], xt_r[:, xg * XG : (xg + 1) * XG, q0 : q0 + 512]
                        )
                        xt_tiles.append(xs)

                qp = [
                    proj_ps.tile([P, 512], f32, name=f"qp{h}", tag=f"qp{h}")
                    for h in range(GQ)
                ]
                kp = proj_ps.tile([P, 512], f32, name="kp", tag="kp")
                vp = proj_ps.tile([P, 512], f32, name="vp", tag="vp")
                for cc in range(NCC):
                    xtile = xt_tiles[cc // XG][:, cc % XG, :]
                    first, last = cc == 0, cc == NCC - 1
                    for h in range(GQ):
                        nc.tensor.matmul(
                            qp[h][:],
                            wq_sb[:, cc, h * D : (h + 1) * D],
                            xtile,
                            start=first,
                            stop=last,
                        )
                    nc.tensor.matmul(
                        kp[:], wk_sb[:, cc, :], xtile, start=first, stop=last
                    )
                    nc.tensor.matmul(
                        vp[:], wv_sb[:, cc, :], xtile, start=first, stop=last
                    )

                cosq = cos_sb[:, q0 : q0 + 512]
                sinq = sin_sb[:, q0 : q0 + 512]

                def rope(pt_ps, dst):
                    # dst = pt*cos + (R pt)*sin ; pt_ps is the PSUM projection
                    raw = ptmp.tile([P, 512], bf16, name="rraw", tag="rraw")
                    nc.scalar.copy(raw[:], pt_ps[:])
                    rp = aux_ps.tile([P, 512], f32, name="rotp", tag="rotp")
                    nc.tensor.matmul(rp[:], rot_sb[:], raw[:], start=True, stop=True)
                    nc.vector.tensor_tensor(dst, raw[:], cosq, MULT)
                    t2 = ptmp.tile([P, 512], bf16, name="rt2", tag="rt2")
                    nc.vector.tensor_tensor(t2[:], rp[:], sinq, MULT)
                    nc.vector.tensor_tensor(dst, dst, t2[:], ADD)

                for h in range(GQ):
                    rope(qp[h], qt_sb[h][:, q0 : q0 + 512])
                rope(kp, kt_sb[:, q0 : q0 + 512])

                # V: evacuate V^T, then PE-transpose to natural [k, D] tiles
                vraw = ptmp.tile([P, 512], bf16, name="vraw", tag="vraw")
                nc.scalar.copy(vraw[:], vp[:])
                for ks in range(4):
                    tp = aux_ps.tile([P, P], bf16, name="vtrp", tag="vtrp")
                    nc.tensor.transpose(tp[:], vraw[:, ks * P : (ks + 1) * P], ident[:])
                    nc.vector.tensor_copy(v_sb[:, qc * 4 + ks, :], tp[:])

        # -------- phase 2: causal attention + interleaved o_proj --------
        with (
            tc.tile_pool(name="pt_pool", bufs=3) as pt_pool,
            tc.tile_pool(name="s_ps", bufs=2, space="PSUM") as s_ps,
            tc.tile_pool(name="y_ps", bufs=1, space="PSUM") as y_ps,
            tc.tile_pool(name="rs_ps", bufs=1, space="PSUM") as rs_ps,
            tc.tile_pool(name="o_ps", bufs=2, space="PSUM") as o_ps,
            tc.tile_pool(name="nrm", bufs=2) as nrm_pool,
            tc.tile_pool(name="ost", bufs=4) as ost_pool,
        ):
            o_count = [0]
            o_queues = (nc.sync, nc.scalar, nc.gpsimd)

            def o_unit(aq, ct, qb):
                # one o_proj output tile [128 q rows, 512 cols] for chunk aq
                op = o_ps.tile([P, 512], f32, name="op", tag="op")
                for h in range(GQ):
                    nc.tensor.matmul(
                        op[:],
                        y_sb[h][:, qb * P : (qb + 1) * P],
                        wo_sb[:, h, ct, :],
                        start=(h == 0),
                        stop=(h == GQ - 1),
                    )
                ot = ost_pool.tile([P, 512], bf16, name="ot", tag="ot")
                nc.vector.tensor_copy(ot[:], op[:])
                oq = o_queues[o_count[0] % 3]
                o_count[0] += 1
                oq.dma_start(
                    out[qb * P : (qb + 1) * P, ct * 512 : (ct + 1) * 512],
                    ot[:],
                )

            def make_units(aq):
                return [(aq, ct, qb) for ct in range(NCT)
                        for qb in range(4 * aq, 4 * aq + 4)]

            for aq in range(NQC):
                q0 = aq * 512
                nks = 4 * (aq + 1)  # 128-wide k subtiles (incl 4 diagonal)
                ng = nks // 2  # groups of 2 subtiles
                units = make_units(aq - 1) if aq > 0 else []
                slots = GQ * ng
                credit = 0.0
                ucount = len(units)

                # narrowed (offset, width) per k-subtile: diagonal subtile m
                # only covers q >= 128m within the 512-wide chunk.
                def ow(ks):
                    m = ks - (nks - 4)
                    if m > 0:
                        return 128 * m, 512 - 128 * m
                    return 0, 512

                for h in range(GQ):
                    qrow = qt_sb[h]
                    yp = y_ps.tile([P, 512], f32, name="yp", tag="yp")
                    rp_ = rs_ps.tile([P, 512], f32, name="rsp", tag="rsp")
                    sps = [None] * ng

                    def s_issue(g):
                        # the two subtiles are packed back to back in the sp
                        # tile ([0:w0], [w0:w0+w1]); w0 is always 256 or 512
                        # so neither matmul output crosses a PSUM bank.
                        sp = s_ps.tile([P, 1024], f32, name="sp", tag="sp")
                        off1 = 0
                        for ks in (2 * g, 2 * g + 1):
                            off, w = ow(ks)
                            nc.tensor.matmul(
                                sp[:, off1 : off1 + w],
                                kt_sb[:, ks * P : (ks + 1) * P],
                                qrow[:, q0 + off : q0 + 512],
                                start=True,
                                stop=True,
                            )
                            off1 += w
                        sps[g] = sp

                    s_issue(0)
                    if ng > 1:
                        s_issue(1)
                    for g in range(ng):
                        if g + 2 < ng:
                            s_issue(g + 2)
                        # o_proj filler for the previous q-chunk
                        credit += ucount / slots
                        while credit >= 1.0 and units:
                            o_unit(*units.pop(0))
                            credit -= 1.0
                        sp = sps[g]
                        pt = pt_pool.tile([P, 1024], bf16, name="ptile", tag="pt")
                        subs = (2 * g, 2 * g + 1)
                        wsum = ow(subs[0])[1] + ow(subs[1])[1]
                        nc.scalar.activation(
                            pt[:, 0:wsum], sp[:, 0:wsum], Exp, scale=SCALE
                        )
                        off1 = 0
                        for ks in subs:
                            w = ow(ks)[1]
                            if ks - (nks - 4) >= 0:
                                # causal triangle on the first 128 cols of
                                # the narrowed slice
                                sl = pt[:, off1 : off1 + P]
                                nc.vector.tensor_tensor(sl, sl, tri_sb[:], MULT)
                            off1 += w
                        off1 = 0
                        for ks in subs:
                            off, w = ow(ks)
                            first, last = ks == 0, ks == nks - 1
                            prhs = pt[:, off1 : off1 + w]
                            off1 += w
                            nc.tensor.matmul(
                                yp[:, off : off + w],
                                v_sb[:, ks, :],
                                prhs,
                                start=first,
                                stop=last,
                            )
                            nc.tensor.matmul(
                                rp_[:, off : off + w],
                                ones_sb[:],
                                prhs,
                                start=first,
                                stop=last,
                            )
                    # 1/rowsum (~18 bits; rowsum >= 1 so no edge cases)
                    rinv = nrm_pool.tile([P, 512], f32, name="rinv", tag="rinv")
                    nc.vector.reciprocal_approx_fast(rinv[:], rp_[:])
                    nc.vector.tensor_tensor(
                        y_sb[h][:, q0 : q0 + 512], yp[:], rinv[:], MULT
                    )
                # drain any leftover filler units of the previous chunk
                for u in units:
                    o_unit(*u)
            # o_proj for the last q-chunk (pure matmul tail, no stalls)
            for u in make_units(NQC - 1):
                o_unit(*u)

    nc.compile()
    return nc


def _bf16(a):
    import ml_dtypes

    return np.ascontiguousarray(np.asarray(a, dtype=np.float32)).astype(
        ml_dtypes.bfloat16
    )


def make_in_maps(x, wq, wk, wv, wo, T=T_FULL):
    """Per-core input dicts for run_bass_kernel_spmd."""
    cosT, sinT = _rope_tables(T)
    tri = _tri128()
    onesm = np.ones((P, P), dtype=np.float32)
    rotm = _rot_lhsT()

    xts = [_bf16(x[b].T) for b in range(B)]
    cosT, sinT, tri, onesm, rotm = map(_bf16, (cosT, sinT, tri, onesm, rotm))
    in_maps = []
    for core in range(NCORES):
        b, g = core // 4, core % 4
        in_maps.append(
            {
                "xt": xts[b],
                "wq": _bf16(wq[:, 512 * g : 512 * (g + 1)]),
                "wk": _bf16(wk[:, D * g : D * (g + 1)]),
                "wv": _bf16(wv[:, D * g : D * (g + 1)]),
                "wo": _bf16(wo[512 * g : 512 * (g + 1), :]),
                "cosT": cosT,
                "sinT": sinT,
                "trim": tri,
                "onesm": onesm,
                "rotm": rotm,
            }
        )
    return in_maps


_NC_CACHE = {}


def _get_nc(T=T_FULL):
    if T not in _NC_CACHE:
        _NC_CACHE[T] = build_nc(T)
    return _NC_CACHE[T]


def run(inputs, trace=False):
    """Run on 8 NeuronCores. Returns (full_output, BassKernelResults)."""
    from concourse.bass_utils import run_bass_kernel_spmd

    x = np.asarray(inputs["x"], dtype=np.float32)
    in_maps = make_in_maps(
        x,
        np.asarray(inputs["wq"], dtype=np.float32),
        np.asarray(inputs["wk"], dtype=np.float32),
        np.asarray(inputs["wv"], dtype=np.float32),
        np.asarray(inputs["wo"], dtype=np.float32),
    )
    nc = _get_nc()
    res = run_bass_kernel_spmd(nc, in_maps, list(range(NCORES)), trace=trace)
    outs = res.results
    full = np.zeros((B, T_FULL, C_DIM), dtype=np.float32)
    for core in range(NCORES):
        full[core // 4] += np.asarray(outs[core]["out"], dtype=np.float32)
    return full, res


def kernel(**inputs):
    full, _ = run(inputs, trace=False)
    return full



# revision 8
# speedup vs baseline: 1.0520x; 1.0520x over previous
"""Trainium2 Bass kernel for causal self-attention with GQA + RoPE.

Model: B=2, T=2048, C=2048, H=16 query heads, H_KV=4 kv heads, D=128.

Sharding (8 NeuronCores, pure SPMD, no collectives):
  core i -> batch b = i // 4, kv-group g = i % 4
            (query heads 4g..4g+3, kv head g, all T positions of batch b).
  Every core runs an identical program; only input data differs.
  o_proj is computed against the row-slice wo[512g:512(g+1), :], giving a
  partial [T, C] output per core; the sum over the 4 cores of each batch
  (the tensor-parallel all-reduce) is done on the host in numpy.

v3 (on top of the bf16 v2 baseline):
  - startup DMA striping: the phase-0 critical bytes (x chunk 0, wq, wk,
    wv; ~4.5MB) are striped per-cc across all four engine DMA queues
    (sync/scalar/vector/gpsimd) in consumption order, so the projection
    stream is dense from ~2us and HAM un-throttles at ~5us (was 16us).
  - rowsum pair-reduce: adjacent 128-wide k-subtiles of exp(S) are summed
    pairwise on DVE (bf16 tensor_tensor, 2x mode); the ones-matmul rowsum
    then streams half the columns (PE 29us -> 16us). Cross-pair
    accumulation stays in PSUM fp32 (the single bf16 pair-add rounds
    independently per element -> rowsum error ~2^-9/sqrt(1024), trivial).
    The final (diagonal) group of each head keeps direct rowsum matmuls
    so no cross-head deferral is needed.
  - o_proj PSUM evacuation alternates vector/scalar (was all-vector, which
    serialized the o_unit pipeline through one engine).
  - tail restructure: the attention-only PSUM pools close before the last
    chunk's o_proj, freeing 6 banks; the tail runs from a 4-deep PSUM pool
    with stores round-robined on sync/scalar/vector. gpsimd (slow ~7.6us
    software-DGE drain) issues no DMA after mid-attention.
"""

import math
import os

import numpy as np

os.environ.setdefault("MYCRO_LOCAL_CACHE", "1")

P = 128
D = 128
H = 16
H_KV = 4
GQ = H // H_KV  # 4 query heads per kv head (= per core)
B = 2
T_FULL = 2048
C_DIM = 2048
NCORES = 8
ROPE_BASE = 10000.0


def _rope_tables(T):
    inv_freq = 1.0 / (ROPE_BASE ** (np.arange(0, D, 2, dtype=np.float32) / D))
    t = np.arange(T, dtype=np.float32)
    freqs = np.outer(t, inv_freq)  # [T, D/2]
    emb = np.concatenate((freqs, freqs), axis=-1)  # [T, D]
    return (
        np.ascontiguousarray(np.cos(emb).T.astype(np.float32)),  # [D, T]
        np.ascontiguousarray(np.sin(emb).T.astype(np.float32)),
    )


def _rot_lhsT():
    # rotate_half(q) = R @ q with R[d, d+64] = -1 (d < 64), R[d, d-64] = +1.
    # matmul computes lhsT.T @ rhs, so pass lhsT = R^T.
    R = np.zeros((D, D), dtype=np.float32)
    half = D // 2
    R[np.arange(half), np.arange(half) + half] = -1.0
    R[np.arange(half) + half, np.arange(half)] = 1.0
    return np.ascontiguousarray(R.T)


def _tri128():
    # tri[k, j] = 1 if j >= k else 0: the in-subtile causal triangle after
    # diagonal narrowing (column j of a narrowed diag slice is q = 128m + j,
    # row k is k_local; valid iff j >= k).
    k = np.arange(P)
    return (k[None, :] >= k[:, None]).astype(np.float32)


def build_nc(T=T_FULL):
    """Build the per-core Bass/Tile program (identical across cores)."""
    from contextlib import ExitStack

    import concourse.mybir as mybir
    import concourse.tile as tile
    from concourse import bacc
    from concourse.masks import make_identity

    f32 = mybir.dt.float32
    bf16 = mybir.dt.bfloat16
    Exp = mybir.ActivationFunctionType.Exp
    MULT = mybir.AluOpType.mult
    ADD = mybir.AluOpType.add
    SCALE = 1.0 / math.sqrt(D)

    NCC = C_DIM // P  # 16 contraction chunks
    NQC = T // 512  # projection / attention q-chunks (512-wide)
    NCT = C_DIM // 512  # o_proj column tiles
    NKB = T // P  # 128-wide k subtiles

    nc = bacc.Bacc(
        "TRN2",
        target_bir_lowering=False,
        debug=False,
        num_devices=NCORES,
    )

    xt = nc.dram_tensor("xt", [C_DIM, T], bf16, kind="ExternalInput").ap()
    wq = nc.dram_tensor("wq", [C_DIM, GQ * D], bf16, kind="ExternalInput").ap()
    wk = nc.dram_tensor("wk", [C_DIM, D], bf16, kind="ExternalInput").ap()
    wv = nc.dram_tensor("wv", [C_DIM, D], bf16, kind="ExternalInput").ap()
    wo = nc.dram_tensor("wo", [GQ * D, C_DIM], bf16, kind="ExternalInput").ap()
    cosT = nc.dram_tensor("cosT", [D, T], bf16, kind="ExternalInput").ap()
    sinT = nc.dram_tensor("sinT", [D, T], bf16, kind="ExternalInput").ap()
    trim = nc.dram_tensor("trim", [P, P], bf16, kind="ExternalInput").ap()
    onesm = nc.dram_tensor("onesm", [P, P], bf16, kind="ExternalInput").ap()
    rotm = nc.dram_tensor("rotm", [P, P], bf16, kind="ExternalInput").ap()
    out = nc.dram_tensor("out", [T, C_DIM], bf16, kind="ExternalOutput").ap()

    with tile.TileContext(nc) as tc, ExitStack() as ctx:
        const = ctx.enter_context(tc.tile_pool(name="const", bufs=1))
        acts = ctx.enter_context(tc.tile_pool(name="acts", bufs=1))

        wq_r = wq.rearrange("(cc p) n -> p cc n", p=P)
        wk_r = wk.rearrange("(cc p) n -> p cc n", p=P)
        wv_r = wv.rearrange("(cc p) n -> p cc n", p=P)
        xt_r = xt.rearrange("(cc p) t -> p cc t", p=P)
        wo_r = wo.rearrange("(h p) (ct n) -> p h ct n", p=P, n=512)

        ones_sb = const.tile([P, P], bf16)
        rot_sb = const.tile([P, P], bf16)
        ident = const.tile([P, P], bf16)
        tri_sb = const.tile([P, P], bf16)

        # long-lived activations (all bf16: 44KB/partition total)
        qt_sb = [acts.tile([P, T], bf16, name=f"qt{h}") for h in range(GQ)]
        kt_sb = acts.tile([P, T], bf16, name="kt")
        v_sb = acts.tile([P, NKB, D], bf16, name="vnat")
        y_sb = [acts.tile([P, T], bf16, name=f"yt{h}") for h in range(GQ)]
        wo_sb = acts.tile([P, GQ, NCT, 512], bf16, name="wo_sb")

        # ---------------- phase 1: projections + rope ----------------
        with (
            tc.tile_pool(name="pwts", bufs=1) as wpool,
            tc.tile_pool(name="xts", bufs=4) as xt_pool,
            tc.tile_pool(name="rope_t", bufs=1) as rope_pool,
            tc.tile_pool(name="proj_ps", bufs=1, space="PSUM") as proj_ps,
            tc.tile_pool(name="aux_ps", bufs=1, space="PSUM") as aux_ps,
            tc.tile_pool(name="ptmp", bufs=2) as ptmp,
        ):
            wq_sb = wpool.tile([P, NCC, GQ * D], bf16)
            wk_sb = wpool.tile([P, NCC, D], bf16)
            wv_sb = wpool.tile([P, NCC, D], bf16)
            lead_xs = xt_pool.tile([P, NCC, 512], bf16, tag="xlead", name="lead_xs")
            cos_sb = rope_pool.tile([P, T], bf16)
            sin_sb = rope_pool.tile([P, T], bf16)

            # identity first: two cheap gpsimd ops, then gpsimd is free to
            # issue DMA descriptors.
            make_identity(nc, ident)

            # Phase-0 critical bytes (x chunk 0 / wq / wk / wv, ~4.5MB)
            # striped per-cc across the three DMA-capable engine queues
            # (sync/scalar hardware-DGE, gpsimd software-DGE) in consumption
            # order: every cc's tensors land on different queues, so delivery
            # (~1.2us/cc aggregate) keeps pace with the PE's warm consumption
            # rate (~1.3us/cc).
            queues = (nc.sync, nc.scalar, nc.gpsimd)
            for cc in range(NCC):
                r = cc % 3
                queues[r].dma_start(lead_xs[:, cc, :], xt_r[:, cc, 0:512])
                queues[(r + 1) % 3].dma_start(wq_sb[:, cc, :], wq_r[:, cc, :])
                queues[(r + 2) % 3].dma_start(wk_sb[:, cc, :], wk_r[:, cc, :])
                queues[(r + 2) % 3].dma_start(wv_sb[:, cc, :], wv_r[:, cc, :])
            # chunk-0 rope tables next on sync (needed ~22us in).
            nc.sync.dma_start(cos_sb[:, 0:512], cosT[:, 0:512])
            nc.sync.dma_start(sin_sb[:, 0:512], sinT[:, 0:512])
            # consts + remaining rope tables (needed later).
            nc.scalar.dma_start(ones_sb[:], onesm)
            nc.scalar.dma_start(rot_sb[:], rotm)
            nc.scalar.dma_start(tri_sb[:], trim)
            for c in range(1, NQC):
                q_ = (nc.sync, nc.scalar)[c % 2]
                q_.dma_start(cos_sb[:, 512 * c : 512 * (c + 1)],
                             cosT[:, 512 * c : 512 * (c + 1)])
                q_.dma_start(sin_sb[:, 512 * c : 512 * (c + 1)],
                             sinT[:, 512 * c : 512 * (c + 1)])
            # wo preload on gpsimd after its striped share (needed ~115us).
            for h in range(GQ):
                for ct in range(NCT):
                    nc.gpsimd.dma_start(wo_sb[:, h, ct, :], wo_r[:, h, ct, :])
            # warm the ACT exp table set during the initial DMA wait
            warm = ptmp.tile([P, 1], f32, name="warm", tag="warm")
            nc.scalar.activation(warm[:], warm[:], Exp)

            XG = 2  # xt c-chunks per streamed tile (chunks 1..3)
            x_queues = (nc.sync, nc.scalar)
            for qc in range(NQC):
                q0 = qc * 512
                if qc == 0:
                    xt_tiles = [lead_xs[:, xg * XG : (xg + 1) * XG, :]
                                for xg in range(NCC // XG)]
                else:
                    xt_tiles = []
                    for xg in range(NCC // XG):
                        xs = xt_pool.tile([P, XG, 512], bf16, tag="xt", name="xs")
                        q_ = x_queues[xg % 2]
                        q_.dma_start(
                            xs[:], xt_r[:, xg * XG : (xg + 1) * XG, q0 : q0 + 512]
                        )
                        xt_tiles.append(xs)

                qp = [
                    proj_ps.tile([P, 512], f32, name=f"qp{h}", tag=f"qp{h}")
                    for h in range(GQ)
                ]
                kp = proj_ps.tile([P, 512], f32, name="kp", tag="kp")
                vp = proj_ps.tile([P, 512], f32, name="vp", tag="vp")
                for cc in range(NCC):
                    xtile = xt_tiles[cc // XG][:, cc % XG, :]
                    first, last = cc == 0, cc == NCC - 1
                    for h in range(GQ):
                        nc.tensor.matmul(
                            qp[h][:],
                            wq_sb[:, cc, h * D : (h + 1) * D],
                            xtile,
                            start=first,
                            stop=last,
                        )
                    nc.tensor.matmul(
                        kp[:], wk_sb[:, cc, :], xtile, start=first, stop=last
                    )
                    nc.tensor.matmul(
                        vp[:], wv_sb[:, cc, :], xtile, start=first, stop=last
                    )

                cosq = cos_sb[:, q0 : q0 + 512]
                sinq = sin_sb[:, q0 : q0 + 512]

                def rope(pt_ps, dst):
                    # dst = pt*cos + (R pt)*sin ; pt_ps is the PSUM projection
                    raw = ptmp.tile([P, 512], bf16, name="rraw", tag="rraw")
                    nc.scalar.copy(raw[:], pt_ps[:])
                    rp = aux_ps.tile([P, 512], f32, name="rotp", tag="rotp")
                    nc.tensor.matmul(rp[:], rot_sb[:], raw[:], start=True, stop=True)
                    nc.vector.tensor_tensor(dst, raw[:], cosq, MULT)
                    t2 = ptmp.tile([P, 512], bf16, name="rt2", tag="rt2")
                    nc.vector.tensor_tensor(t2[:], rp[:], sinq, MULT)
                    nc.vector.tensor_tensor(dst, dst, t2[:], ADD)

                for h in range(GQ):
                    rope(qp[h], qt_sb[h][:, q0 : q0 + 512])
                rope(kp, kt_sb[:, q0 : q0 + 512])

                # V: evacuate V^T, then PE-transpose to natural [k, D] tiles
                vraw = ptmp.tile([P, 512], bf16, name="vraw", tag="vraw")
                nc.scalar.copy(vraw[:], vp[:])
                for ks in range(4):
                    tp = aux_ps.tile([P, P], bf16, name="vtrp", tag="vtrp")
                    nc.tensor.transpose(tp[:], vraw[:, ks * P : (ks + 1) * P], ident[:])
                    nc.vector.tensor_copy(v_sb[:, qc * 4 + ks, :], tp[:])

        # -------- phase 2: causal attention + interleaved o_proj --------
        with (
            tc.tile_pool(name="pt_pool", bufs=3) as pt_pool,
            tc.tile_pool(name="o_ps", bufs=2, space="PSUM") as o_ps,
            tc.tile_pool(name="nrm", bufs=2) as nrm_pool,
            tc.tile_pool(name="ost", bufs=4) as ost_pool,
            tc.tile_pool(name="pairs", bufs=3) as pair_pool,
        ):
            o_count = [0]
            o_queues = (nc.sync, nc.scalar, nc.gpsimd)
            evac_engines = (nc.vector, nc.scalar)

            def o_unit(aq, ct, qb, ps_pool, store_queues):
                # one o_proj output tile [128 q rows, 512 cols] for chunk aq
                op = ps_pool.tile([P, 512], f32, name="op", tag="op")
                for h in range(GQ):
                    nc.tensor.matmul(
                        op[:],
                        y_sb[h][:, qb * P : (qb + 1) * P],
                        wo_sb[:, h, ct, :],
                        start=(h == 0),
                        stop=(h == GQ - 1),
                    )
                ot = ost_pool.tile([P, 512], bf16, name="ot", tag="ot")
                ev = evac_engines[o_count[0] % 2]
                if ev is nc.scalar:
                    nc.scalar.copy(ot[:], op[:])
                else:
                    nc.vector.tensor_copy(ot[:], op[:])
                oq = store_queues[o_count[0] % len(store_queues)]
                o_count[0] += 1
                oq.dma_start(
                    out[qb * P : (qb + 1) * P, ct * 512 : (ct + 1) * 512],
                    ot[:],
                )

            def make_units(aq):
                return [(aq, ct, qb) for ct in range(NCT)
                        for qb in range(4 * aq, 4 * aq + 4)]

            with (
                tc.tile_pool(name="s_ps", bufs=2, space="PSUM") as s_ps,
                tc.tile_pool(name="y_ps", bufs=1, space="PSUM") as y_ps,
                tc.tile_pool(name="rs_ps", bufs=1, space="PSUM") as rs_ps,
            ):
                for aq in range(NQC):
                    q0 = aq * 512
                    nks = 4 * (aq + 1)  # 128-wide k subtiles (incl 4 diagonal)
                    ng = nks // 2  # groups of 2 subtiles
                    units = make_units(aq - 1) if aq > 0 else []
                    slots = GQ * ng
                    credit = 0.0
                    ucount = len(units)

                    # narrowed (offset, width) per k-subtile: diagonal subtile
                    # m only covers q >= 128m within the 512-wide chunk.
                    def ow(ks):
                        m = ks - (nks - 4)
                        if m > 0:
                            return 128 * m, 512 - 128 * m
                        return 0, 512

                    for h in range(GQ):
                        qrow = qt_sb[h]
                        yp = y_ps.tile([P, 512], f32, name="yp", tag="yp")
                        rp_ = rs_ps.tile([P, 512], f32, name="rsp", tag="rsp")
                        sps = [None] * ng
                        # pair tiles awaiting their rowsum matmul:
                        # list of (tile, offA) in group order
                        pend_pairs = [None] * ng

                        def s_issue(g):
                            # the two subtiles are packed back to back in the
                            # sp tile ([0:w0], [w0:w0+w1]); w0 is always 256
                            # or 512 so neither matmul output crosses a PSUM
                            # bank.
                            sp = s_ps.tile([P, 1024], f32, name="sp", tag="sp")
                            off1 = 0
                            for ks in (2 * g, 2 * g + 1):
                                off, w = ow(ks)
                                nc.tensor.matmul(
                                    sp[:, off1 : off1 + w],
                                    kt_sb[:, ks * P : (ks + 1) * P],
                                    qrow[:, q0 + off : q0 + 512],
                                    start=True,
                                    stop=True,
                                )
                                off1 += w
                            sps[g] = sp

                        s_issue(0)
                        if ng > 1:
                            s_issue(1)
                        for g in range(ng):
                            if g + 2 < ng:
                                s_issue(g + 2)
                            # rowsum matmul for the PREVIOUS group's pair:
                            # emitted before this group's PV/direct matmuls
                            # so pair 0 (start=True) is always rp_'s first
                            # writer; its DVE add has had ~a full group to
                            # finish.
                            if g >= 1 and pend_pairs[g - 1] is not None:
                                pr, poff = pend_pairs[g - 1]
                                nc.tensor.matmul(
                                    rp_[:, poff:512],
                                    ones_sb[:],
                                    pr[:, poff:512],
                                    start=(g - 1 == 0),
                                    stop=False,
                                )
                                pend_pairs[g - 1] = None
                            # o_proj filler for the previous q-chunk
                            credit += ucount / slots
                            while credit >= 1.0 and units:
                                o_unit(*units.pop(0), o_ps, o_queues)
                                credit -= 1.0
                            sp = sps[g]
                            pt = pt_pool.tile([P, 1024], bf16, name="ptile",
                                              tag="pt")
                            subs = (2 * g, 2 * g + 1)
                            (offA, wA), (offB, wB) = ow(subs[0]), ow(subs[1])
                            wsum = wA + wB
                            nc.scalar.activation(
                                pt[:, 0:wsum], sp[:, 0:wsum], Exp, scale=SCALE
                            )
                            off1 = 0
                            for ks in subs:
                                w = ow(ks)[1]
                                if ks - (nks - 4) >= 0:
                                    # causal triangle on the first 128 cols
                                    # of the narrowed slice
                                    sl = pt[:, off1 : off1 + P]
                                    nc.vector.tensor_tensor(sl, sl, tri_sb[:],
                                                            MULT)
                                off1 += w
                            last_group = g == ng - 1
                            if not last_group:
                                # pair-reduce the two subtiles on DVE (bf16,
                                # one rounding per element); the rowsum
                                # matmul on the pair streams half the cols.
                                pair = pair_pool.tile([P, 512], bf16,
                                                      name="pair", tag="pair")
                                if offB > offA:
                                    # diagonal pair: [offA:offB] has only A
                                    nc.vector.tensor_copy(
                                        pair[:, offA:offB],
                                        pt[:, 0 : offB - offA],
                                    )
                                    nc.vector.tensor_tensor(
                                        pair[:, offB:512],
                                        pt[:, offB - offA : wA],
                                        pt[:, wA : wA + wB],
                                        ADD,
                                    )
                                else:
                                    nc.vector.tensor_tensor(
                                        pair[:, 0:512],
                                        pt[:, 0:512],
                                        pt[:, 512:1024],
                                        ADD,
                                    )
                                pend_pairs[g] = (pair, offA)
                            off1 = 0
                            for ks in subs:
                                off, w = ow(ks)
                                first, last = ks == 0, ks == nks - 1
                                prhs = pt[:, off1 : off1 + w]
                                off1 += w
                                nc.tensor.matmul(
                                    yp[:, off : off + w],
                                    v_sb[:, ks, :],
                                    prhs,
                                    start=first,
                                    stop=last,
                                )
                                if last_group:
                                    # final (diagonal) group: direct rowsum
                                    # matmuls (executed after pair 0's
                                    # start=True matmul) so nothing is
                                    # deferred across the head boundary.
                                    nc.tensor.matmul(
                                        rp_[:, off : off + w],
                                        ones_sb[:],
                                        prhs,
                                        start=False,
                                        stop=(ks == nks - 1),
                                    )
                        # 1/rowsum (~18 bits; rowsum >= 1 so no edge cases)
                        rinv = nrm_pool.tile([P, 512], f32, name="rinv",
                                             tag="rinv")
                        nc.vector.reciprocal_approx_fast(rinv[:], rp_[:])
                        nc.vector.tensor_tensor(
                            y_sb[h][:, q0 : q0 + 512], yp[:], rinv[:], MULT
                        )
                    # drain any leftover filler units of the previous chunk
                    for u in units:
                        o_unit(*u, o_ps, o_queues)
            # attention PSUM pools closed: 6 banks free. o_proj tail for the
            # last q-chunk runs from a 4-deep PSUM pool (pure matmul stream;
            # evacuation fully hidden), stores on sync/scalar only (gpsimd
            # issues nothing this late - its software-DGE drain is ~7.6us).
            tail_queues = (nc.sync, nc.scalar)
            with tc.tile_pool(name="o_tail_ps", bufs=4, space="PSUM") as o_tail:
                for u in make_units(NQC - 1):
                    o_unit(*u, o_tail, tail_queues)

    nc.compile()
    return nc


def _bf16(a):
    import ml_dtypes

    return np.ascontiguousarray(np.asarray(a, dtype=np.float32)).astype(
        ml_dtypes.bfloat16
    )


def make_in_maps(x, wq, wk, wv, wo, T=T_FULL):
    """Per-core input dicts for run_bass_kernel_spmd."""
    cosT, sinT = _rope_tables(T)
    tri = _tri128()
    onesm = np.ones((P, P), dtype=np.float32)
    rotm = _rot_lhsT()

    xts = [_bf16(x[b].T) for b in range(B)]
    cosT, sinT, tri, onesm, rotm = map(_bf16, (cosT, sinT, tri, onesm, rotm))
    in_maps = []
    for core in range(NCORES):
        b, g = core // 4, core % 4
        in_maps.append(
            {
                "xt": xts[b],
                "wq": _bf16(wq[:, 512 * g : 512 * (g + 1)]),
                "wk": _bf16(wk[:, D * g : D * (g + 1)]),
                "wv": _bf16(wv[:, D * g : D * (g + 1)]),
                "wo": _bf16(wo[512 * g : 512 * (g + 1), :]),
                "cosT": cosT,
                "sinT": sinT,
                "trim": tri,
                "onesm": onesm,
                "rotm": rotm,
            }
        )
    return in_maps


_NC_CACHE = {}


def _get_nc(T=T_FULL):
    if T not in _NC_CACHE:
        _NC_CACHE[T] = build_nc(T)
    return _NC_CACHE[T]


def run(inputs, trace=False):
    """Run on 8 NeuronCores. Returns (full_output, BassKernelResults)."""
    from concourse.bass_utils import run_bass_kernel_spmd

    x = np.asarray(inputs["x"], dtype=np.float32)
    in_maps = make_in_maps(
        x,
        np.asarray(inputs["wq"], dtype=np.float32),
        np.asarray(inputs["wk"], dtype=np.float32),
        np.asarray(inputs["wv"], dtype=np.float32),
        np.asarray(inputs["wo"], dtype=np.float32),
    )
    nc = _get_nc()
    res = run_bass_kernel_spmd(nc, in_maps, list(range(NCORES)), trace=trace)
    outs = res.results
    full = np.zeros((B, T_FULL, C_DIM), dtype=np.float32)
    for core in range(NCORES):
        full[core // 4] += np.asarray(outs[core]["out"], dtype=np.float32)
    return full, res


def kernel(**inputs):
    full, _ = run(inputs, trace=False)
    return full
